# revision 23
# baseline (speedup 1.0000x reference)
"""TRN2 Bass kernel for nn_Brain: delayed-synapse recurrent network.

Strategy (dense delay-batched "futures"):
  total_input[t] = c0 + sum_{d=1}^{15} W_d @ acts_{t-d}   (acts_s, s>=1)
  acts_t = tanh(total_input[t])
- Edges with delay >= 16 never fire (valid = delay < t <= 16): dropped.
- delay-0 edges always read hist[0] (python history[-0] quirk) => per-neuron
  constant c0, computed on host from the input row.
- W_d stored dense [4096 src, 512 tgt] fp8e4m3 (x64 scale, unscaled in the
  drain op) per core (8-way target shard); both batch rows ride the same
  weight stream as extra matmul columns.
- Bucket d may batch up to d consecutive steps in one application (the
  contribution to step t uses acts_{t-d}, available d-1 steps early), so
  bucket d is applied ceil((16-d)/d) times instead of 16-d. d=1..7 stay
  SBUF-resident; d>=8 stream from HBM with a 5-deep prefetch pool.
- Each application accumulates over source chunks in a private ping-pong
  PSUM scratch, then drains into an SBUF fp32 accumulator (one DVE op).
  Per step: tanh (ScalarE) straight from the accumulator, AllGather the
  512-target slice across the 8 cores via DRAM bounce, land it directly
  into the SBUF activation-history tile that feeds later matmuls.
"""
import numpy as np

N_NEURONS = 4096
INPUT_SIZE = 1024
BATCH = 2
STEPS = 16
N_CORES = 8
TGT_PER_CORE = N_NEURONS // N_CORES        # 512
TCH = TGT_PER_CORE // 128                  # 4 target chunks per core
SCH = N_NEURONS // 128                     # 32 source chunks
MAXD = STEPS - 1                           # delays 1..15 useful
RESIDENT_D = (1, 2, 3, 4, 5, 6, 7)
FP8_SCALE = 64.0

_compiled = None


def _schedule():
    """Apps: (d, s0, nb) -> contributes to steps t in [s0+d, s0+d+nb-1]
    using acts_{s0..s0+nb-1} (nb <= d, windows balanced per bucket)."""
    apps = []
    for d in range(1, MAXD + 1):
        nsteps = STEPS - d           # t = d+1..16 -> s = 1..16-d
        nwin = -(-nsteps // d)       # ceil
        base, extra = divmod(nsteps, nwin)
        s0 = 1
        for i in range(nwin):
            nb = base + (1 if i < extra else 0)
            apps.append((d, s0, nb))
            s0 += nb
    return apps


def _build_program():
    from concourse import bacc, mybir, tile

    dt = mybir.dt
    nc = bacc.Bacc(None, target_bir_lowering=False, debug=False)

    # ---- dram params (identical program on all cores; data differs) ----
    wd_in = {}
    for d in range(1, MAXD + 1):
        wd_in[d] = nc.declare_dram_parameter(
            f"wd{d}", [128, SCH * TCH * 128], dt.float8e4, isOutput=False)
    c0r_in = nc.declare_dram_parameter("c0rep", [128, TCH * STEPS * BATCH],
                                       dt.float32, isOutput=False)
    out_d = nc.declare_dram_parameter("out", [128, TCH * BATCH], dt.float32,
                                      isOutput=True)

    # collective bounce buffers (internal DRAM; shared out for allgather)
    cc_in = nc.dram_tensor("cc_in", [128, TCH * BATCH], dt.bfloat16)
    cc_out = nc.dram_tensor("cc_out", [N_CORES * 128, TCH * BATCH],
                            dt.bfloat16, addr_space="Shared")

    apps = _schedule()
    # Issuance: small buckets as early as their acts allow; big streamed
    # buckets (d>=8, single window) wait until shortly before first use so
    # the early steps aren't congested and program order matches the
    # stream-prefetch order.
    ready = {s: [] for s in range(0, STEPS + 1)}
    for (d, s0, nb) in apps:
        k = s0 + nb - 1          # earliest: all acts available
        if d >= 8:
            k = max(k, s0 + d - 2)   # big buckets: wait until near first use
        ready[k].append((d, s0, nb))

    HCOLS = MAXD * SCH * BATCH  # acts_hist free cols: (s-1, c, r)

    with tile.TileContext(nc) as tc:
        with (
            tc.tile_pool(name="wres", bufs=1) as wres_pool,
            tc.tile_pool(name="wstream", bufs=5) as wstream_pool,
            tc.tile_pool(name="aux", bufs=1) as aux_pool,
            tc.tile_pool(name="psum", bufs=2, space="PSUM") as psum_pool,
        ):
            # resident weights d=1..7: [128, SCH*TCH*128] fp8 (2MB each)
            t_wres = {}
            for d in RESIDENT_D:
                t_wres[d] = wres_pool.tile([128, SCH * TCH * 128], dt.float8e4,
                                           name=f"wres{d}", tag=f"wres{d}")
            t_acc = aux_pool.tile([128, TCH * STEPS * BATCH], dt.float32)
            t_hist = aux_pool.tile([128, HCOLS], dt.bfloat16)
            t_act = aux_pool.tile([128, TCH * BATCH], dt.float32)
            t_actb = aux_pool.tile([128, TCH * BATCH], dt.bfloat16)
            MAXB = 8

            # loads (c0 pre-replicated over step columns -> accumulator init).
            # Only wres1 up front; other resident buckets stagger into the
            # step loop (emitted before any app that reads them) so the
            # first exchanges don't queue behind 14MB of weight DMA.
            nc.sync.dma_start(t_acc[:], c0r_in[:])
            nc.sync.dma_start(t_wres[1][:], wd_in[1][:])
            first_ready = {}
            for (d_, s0_, nb_) in apps:
                first_ready.setdefault(d_, s0_ + nb_ - 1)
            wres_load_at = {d_: min(d_ - 1, first_ready[d_])
                            for d_ in RESIDENT_D if d_ >= 2}



            def run_app(d, s0, nb):
                t_scr = psum_pool.tile([128, TCH * MAXB * BATCH], dt.float32,
                                       name="scr", tag="scr")
                scr4 = t_scr[:].rearrange("p (tcch b r) -> p tcch b r",
                                          tcch=TCH, r=BATCH)
                if d in RESIDENT_D:
                    t_w = t_wres[d]
                else:
                    t_w = wstream_pool.tile([128, SCH * TCH * 128],
                                            dt.float8e4, name="wstream",
                                            tag="wstream")
                    nc.sync.dma_start(t_w[:], wd_in[d][:])
                w3 = t_w[:].rearrange("p (sc tcch m) -> p sc tcch m",
                                      sc=SCH, tcch=TCH)
                t0 = s0 + d
                for tc_i in range(TCH):
                    for sc in range(SCH):
                        rhs = t_hist[:].rearrange(
                            "p (s c r) -> p s c r", s=MAXD, c=SCH
                        )[:, s0 - 1:s0 - 1 + nb, sc, :]
                        nc.tensor.matmul(
                            scr4[:, tc_i, :nb, :], w3[:, sc, tc_i, :], rhs,
                            start=(sc == 0), stop=(sc == SCH - 1))
                # drain scratch into SBUF accumulator (one DVE op)
                acc4 = t_acc[:].rearrange("p (tcch t r) -> p tcch t r",
                                          tcch=TCH, t=STEPS)
                acc_win = acc4[:, :, t0 - 1:t0 - 1 + nb, :]
                nc.vector.scalar_tensor_tensor(
                    acc_win, scr4[:, :, :nb, :], 1.0 / FP8_SCALE, acc_win,
                    mybir.AluOpType.mult, mybir.AluOpType.add)

            for t in range(1, STEPS + 1):
                # epilogue for step t: all apps contributing to t were
                # issued in earlier iterations; Tile's dependency tracking
                # orders the accumulator read after their drains.
                sc_ctx = nc.named_scope(f"step{t:02d}")
                sc_ctx.__enter__()
                acc_t = t_acc[:].rearrange(
                    "p (tcch tt r) -> p tcch tt r", tcch=TCH, tt=STEPS
                )[:, :, t - 1, :]
                nc.scalar.activation(
                    t_actb[:].rearrange("p (tcch r) -> p tcch r", tcch=TCH),
                    acc_t, mybir.ActivationFunctionType.Tanh)
                if t == STEPS:
                    nc.scalar.activation(
                        t_act[:].rearrange("p (tcch r) -> p tcch r", tcch=TCH),
                        acc_t, mybir.ActivationFunctionType.Tanh)
                    nc.sync.dma_start(out_d[:], t_act[:])
                    sc_ctx.__exit__(None, None, None)
                    break
                # allgather acts_t slices across 8 cores
                nc.sync.dma_start(cc_in[:], t_actb[:])
                nc.gpsimd.collective_compute(
                    "AllGather", mybir.AluOpType.bypass,
                    replica_groups=[list(range(N_CORES))],
                    ins=[cc_in[:]], outs=[cc_out[:]])
                # land into history: hist[p, (s=t, c=4j+tc, r)]
                src_ap = cc_out[:].rearrange(
                    "(j p) (tcch r) -> p j tcch r", p=128, r=BATCH)
                dst_ap = t_hist[:].rearrange(
                    "p (s c r) -> p s c r", s=MAXD, c=SCH
                )[:, t - 1, :, :].rearrange(
                    "p (j tcch) r -> p j tcch r", j=N_CORES)
                nc.sync.dma_start(dst_ap, src_ap)
                sc_ctx.__exit__(None, None, None)
                # staggered resident loads (always before apps that use them)
                for d in RESIDENT_D:
                    if wres_load_at.get(d) == t:
                        nc.sync.dma_start(t_wres[d][:], wd_in[d][:])
                # issue apps that became ready with acts_t
                for (d, s0, nb) in ready.get(t, []):
                    with nc.named_scope(f"app_d{d}_s{s0}"):
                        run_app(d, s0, nb)

    nc.compile()
    return nc


def _preprocess(input_data, connection_weights, connection_indices,
                delay_values, steps):
    """Host: build per-core dense bucketed weights, c0, initial acts."""
    assert steps == STEPS
    w = np.asarray(connection_weights, np.float32)
    ci = np.asarray(connection_indices)
    dl = np.asarray(delay_values)
    src, tgt = ci[0].astype(np.int64), ci[1].astype(np.int64)
    x = np.asarray(input_data, np.float32)           # [BATCH, 1024]

    acts0 = np.zeros((BATCH, N_NEURONS), np.float32)
    acts0[:, :INPUT_SIZE] = x

    # c0: delay-0 edges always read acts0[src]
    m0 = dl == 0
    c0 = np.zeros((BATCH, N_NEURONS), np.float32)
    for r in range(BATCH):
        np.add.at(c0[r], tgt[m0], w[m0] * acts0[r, src[m0]])

    # dense W_d [4096 src, 4096 tgt] fp32 per bucket (duplicates summed)
    import ml_dtypes
    wds = {}
    for d in range(1, MAXD + 1):
        md = dl == d
        Wd = np.zeros((N_NEURONS, N_NEURONS), np.float32)
        np.add.at(Wd, (src[md], tgt[md]), w[md])
        wds[d] = Wd

    in_maps = []
    for k in range(N_CORES):
        t0, t1 = k * TGT_PER_CORE, (k + 1) * TGT_PER_CORE
        im = {}
        for d in range(1, MAXD + 1):
            Ws = wds[d][:, t0:t1]                      # [4096, 512]
            # [(sc p), (tc m)] -> [p, (sc, tc, m)]
            Wr = Ws.reshape(SCH, 128, TCH, 128).transpose(1, 0, 2, 3)
            Wr = np.ascontiguousarray(Wr.reshape(128, SCH * TCH * 128))
            im[f"wd{d}"] = (Wr * FP8_SCALE).astype(ml_dtypes.float8_e4m3fn)
        # c0rep[p, (tc, t, r)] = c0[r, target(p, tc)] for every step t
        c0r = np.zeros((128, TCH, STEPS, BATCH), np.float32)
        for tci in range(TCH):
            for r in range(BATCH):
                c0r[:, tci, :, r] = c0[r, t0 + tci * 128:
                                       t0 + (tci + 1) * 128][:, None]
        im["c0rep"] = c0r.reshape(128, TCH * STEPS * BATCH)
        in_maps.append(im)
    return in_maps


def kernel(input_data, connection_weights, connection_indices,
           delay_values, steps):
    global _compiled
    from concourse.bass_utils import run_bass_kernel_spmd

    in_maps = _preprocess(input_data, connection_weights,
                          connection_indices, delay_values, int(steps))
    if _compiled is None:
        _compiled = _build_program()
    res = run_bass_kernel_spmd(_compiled, in_maps, list(range(N_CORES)))

    out = np.zeros((BATCH, N_NEURONS), np.float32)
    for k in range(N_CORES):
        o = res.results[k]["out"]                      # [128, (tc, r)]
        t0 = k * TGT_PER_CORE
        for tci in range(TCH):
            for r in range(BATCH):
                out[r, t0 + tci * 128: t0 + (tci + 1) * 128] = \
                    o[:, tci * BATCH + r]
    return out[:, -INPUT_SIZE:].astype(np.float32)
